# revision 7
# baseline (speedup 1.0000x reference)
"""Mixtral sparse MoE block on 8 Trainium2 NeuronCores.

Strategy: grouped expert parallelism with intermediate-dim slicing.
Experts are snake-packed by token count into NG = 8//M groups of M
experts; each group owns M cores, and each core computes ALL M experts
of its group over an I/M slice of the intermediate dim. Block widths
(token counts per expert, position-wise max across groups, rounded to
8) are compile-time constants, so per-core work is Sum(widths) x
3*2*H*(I/M) FLOPs — balanced across cores and nearly padding-free —
while per-core weight DMA stays at 12.6 MB (same as plain expert
parallelism). The host runs the router / dispatch / combine (adds the
M partial outputs per group, then weighted scatter-add).

Device layout per core (features on partitions, tokens on free dim):
  up[i,t]   = sum_h W1[h,i] * xT[h,t]     i in this core's I/M slice
  gate[i,t] = sum_h W3[h,i] * xT[h,t]
  act[i,t]  = silu(up) * gate             (ACT silu + DVE mul -> bf16)
  out[h,t]  = sum_i W2[i,h] * act[i,t]    partial over the I/M slice

DMA queues: sync streams W1/W3 (+ x blocks 1..M-1) in consumption
order; gpsimd streams W2; scalar loads x block 0 then issues output
DMAs; vector only memsets the PE-warmup tile so warmup starts right
after the preamble instead of waiting on any DMA.
"""

import numpy as np
import ml_dtypes

import bass_rust
import concourse.bass as bass
import concourse.mybir as mybir
import concourse.tile as tile
from concourse.bass_utils import run_bass_kernel_spmd


def _enforce_single_wait(nc):
    """The walrus in this image rejects >1 sync-wait per instruction
    ("Too many sync wait commands", CoreV3GenImpl setupSyncWait). Hoist
    extra waits onto same-engine nops inserted just before the offender
    — waiting earlier on the same sequencer is always safe."""
    for f in nc.m.functions:
        for bb in f.blocks:
            insts = bb.instructions
            i = 0
            while i < len(insts):
                inst = insts[i]
                si = inst.sync_info
                if si is not None and len(si.on_wait) > 1:
                    waits = list(si.on_wait)
                    if any(w.wait_reg is not None for w in waits):
                        i += 1
                        continue
                    for j, w in enumerate(waits[:-1]):
                        nop = mybir.InstNoOp(
                            name=f"{inst.name}_hw{j}", ins=[], outs=[])
                        nop.engine = inst.engine
                        nop.sync_info = bass_rust.SyncInfo(
                            on_wait=[w], on_update=[])
                        insts.insert(i, nop)
                        i += 1
                    inst.sync_info = bass_rust.SyncInfo(
                        on_wait=[waits[-1]], on_update=list(si.on_update))
                i += 1

P = 128
H = 1024
I = 2048
E = 8
K = 2
M = 4            # I-dim slices per expert == experts per group
NG = E // M      # groups (sets of M cores)
I_LOC = I // M   # intermediate channels per core per expert
IT_LOC = I_LOC // P
HK = H // P
WARM_N = 10      # PE warmup matmuls (bridge preamble -> first real matmul)

BF16 = mybir.dt.bfloat16
F32 = mybir.dt.float32

# Populated by the last kernel() call so a harness can inspect HW timing.
LAST_RESULTS = None

_NC_CACHE = {}


def _t_chunks(w):
    """Split a block's token free-dim into matmul chunks <= 512 (PSUM bank)."""
    if w <= 512:
        return [(0, w)]
    half = (w + 1) // 2
    half = (half + 31) // 32 * 32
    return [(0, half), (half, w - half)]


def _build_nc(widths):
    """SPMD program: M expert blocks (token widths `widths`), each an
    I/M-slice SwiGLU MLP; runs on all 8 cores with per-core data."""
    t_tot = sum(widths)
    boffs = [0]
    for w in widths[:-1]:
        boffs.append(boffs[-1] + w)

    nc = bass.Bass()
    xT = nc.declare_dram_parameter("xT", [H, t_tot], BF16, isOutput=False)
    w1 = nc.declare_dram_parameter("w1", [H, I_LOC * M], BF16, isOutput=False)
    w3 = nc.declare_dram_parameter("w3", [H, I_LOC * M], BF16, isOutput=False)
    w2 = nc.declare_dram_parameter("w2", [I_LOC * M, H], BF16, isOutput=False)
    outT = nc.declare_dram_parameter("outT", [H, t_tot], BF16, isOutput=True)

    with tile.TileContext(nc) as tc:
        with (
            tc.tile_pool(name="x", bufs=1) as xpool,
            tc.tile_pool(name="wu", bufs=1) as wupool,
            tc.tile_pool(name="wg", bufs=1) as wgpool,
            tc.tile_pool(name="wd", bufs=1) as wdpool,
            tc.tile_pool(name="acts", bufs=1) as actpool,
            tc.tile_pool(name="warm", bufs=1) as warmpool,
            tc.tile_pool(name="ost", bufs=1) as ostpool,
            tc.tile_pool(name="ps", bufs=2, space="PSUM") as pspool,
            tc.tile_pool(name="ev", bufs=3) as evpool,
        ):
            # Warmup tile: memset on the (otherwise idle) vector queue so
            # PE warmup needs no DMA and starts right after the preamble.
            warm_sb = warmpool.tile([P, 512], BF16, tag="warm", name="warm")
            nc.vector.memset(warm_sb[:], 0.0)

            x_sb = [xpool.tile([P, t_tot], BF16, tag=f"x{hk}", name=f"x{hk}")
                    for hk in range(HK)]
            w1_sb = [wupool.tile([P, I_LOC * M], BF16, tag=f"w1_{hk}",
                                 name=f"w1_{hk}") for hk in range(HK)]
            w3_sb = [wgpool.tile([P, I_LOC * M], BF16, tag=f"w3_{hk}",
                                 name=f"w3_{hk}") for hk in range(HK)]
            w2_sb = [wdpool.tile([P, H], BF16, tag=f"w2_{it}", name=f"w2_{it}")
                     for it in range(IT_LOC * M)]

            # Head (deadline-ordered streaming; HBM aggregate ~360 GB/s is
            # the binding constraint, so only block-0 prerequisites move
            # first): x block 0 split scalar/vector, W1 block 0 on sync and
            # W3 block 0 on gpsimd in 256-col chunks so the first i-tiles
            # unblock after ~half the block-0 weights.
            b0 = slice(boffs[0], boffs[0] + widths[0])
            for hk in range(HK):
                hs = slice(hk * P, (hk + 1) * P)
                nc.scalar.dma_start(out=x_sb[hk][:, b0], in_=xT[hs, b0])
            for c in (slice(0, 256), slice(256, I_LOC)):
                for hk in range(HK):
                    hs = slice(hk * P, (hk + 1) * P)
                    nc.sync.dma_start(out=w1_sb[hk][:, c], in_=w1[hs, c])
            for c in (slice(0, 256), slice(256, I_LOC)):
                for hk in range(HK):
                    hs = slice(hk * P, (hk + 1) * P)
                    nc.gpsimd.dma_start(out=w3_sb[hk][:, c], in_=w3[hs, c])

            # sync queue then streams x/W1/W3 for later blocks in
            # consumption order (each bundle lands >=7us before its phase).
            for b in range(1, M):
                bs = slice(boffs[b], boffs[b] + widths[b])
                cs = slice(b * I_LOC, (b + 1) * I_LOC)
                for hk in range(HK):
                    hs = slice(hk * P, (hk + 1) * P)
                    nc.sync.dma_start(out=x_sb[hk][:, bs], in_=xT[hs, bs])
                for hk in range(HK):
                    hs = slice(hk * P, (hk + 1) * P)
                    nc.sync.dma_start(out=w1_sb[hk][:, cs], in_=w1[hs, cs])
                for hk in range(HK):
                    hs = slice(hk * P, (hk + 1) * P)
                    nc.sync.dma_start(out=w3_sb[hk][:, cs], in_=w3[hs, cs])

            # gpsimd queue: W2 after W3 block 0 — first needed only when
            # phase B of block 0 starts (~25us in), done well before that.
            for it in range(IT_LOC * M):
                nc.gpsimd.dma_start(
                    out=w2_sb[it][:], in_=w2[it * P:(it + 1) * P, :])

            act_sb = [[actpool.tile([P, widths[b]], BF16, tag=f"a{b}_{it}",
                                    name=f"a{b}_{it}")
                       for it in range(IT_LOC)] for b in range(M)]

            # PE warmup on the memset tile: flips the HAM clock gate to 8/8
            # and bridges until the first weights/x arrive (~11us).
            for wi in range(WARM_N):
                w_ps = pspool.tile([P, 512], F32, tag="warm", bufs=1,
                                   name=f"warm{wi}")
                nc.tensor.matmul(
                    w_ps[:], warm_sb[:, 0:P], warm_sb[:],
                    start=True, stop=True)

            for b in range(M):
                wb = widths[b]
                bo = boffs[b]
                chunks = _t_chunks(wb)
                # Phase A: up/gate matmuls + fused silu*gate eviction.
                for it in range(IT_LOC):
                    isl = slice(b * I_LOC + it * P, b * I_LOC + (it + 1) * P)
                    for (t0, tn) in chunks:
                        tsl = slice(bo + t0, bo + t0 + tn)
                        up_ps = pspool.tile([P, tn], F32, tag="up",
                                            name=f"up{b}_{it}_{t0}")
                        gt_ps = pspool.tile([P, tn], F32, tag="gt",
                                            name=f"gt{b}_{it}_{t0}")
                        for hk in range(HK):
                            nc.tensor.matmul(
                                up_ps[:], w1_sb[hk][:, isl], x_sb[hk][:, tsl],
                                start=(hk == 0), stop=(hk == HK - 1))
                        for hk in range(HK):
                            nc.tensor.matmul(
                                gt_ps[:], w3_sb[hk][:, isl], x_sb[hk][:, tsl],
                                start=(hk == 0), stop=(hk == HK - 1))
                        silu_t = evpool.tile([P, tn], F32, tag="silu",
                                             name=f"silu{b}_{it}_{t0}")
                        nc.scalar.activation(
                            silu_t[:], up_ps[:],
                            mybir.ActivationFunctionType.Silu)
                        nc.vector.tensor_mul(
                            act_sb[b][it][:, t0:t0 + tn], silu_t[:], gt_ps[:])

                # Phase B: down projection; each o_ps is evicted by scalar
                # and vector in parallel halves (bf16), then DMA'd on the
                # sync queue (idle once the weight stream is done).
                for h in range(HK):
                    hsl = slice(h * P, (h + 1) * P)
                    o_sb = ostpool.tile([P, wb], BF16, tag=f"ost{b}", bufs=3,
                                        name=f"ost{b}_{h}")
                    for (t0, tn) in chunks:
                        o_ps = pspool.tile([P, tn], F32, tag="o", bufs=3,
                                           name=f"o{b}_{h}_{t0}")
                        for it in range(IT_LOC):
                            nc.tensor.matmul(
                                o_ps[:],
                                w2_sb[b * IT_LOC + it][:, hsl],
                                act_sb[b][it][:, t0:t0 + tn],
                                start=(it == 0), stop=(it == IT_LOC - 1))
                        th = (tn + 15) // 16 * 8
                        nc.scalar.copy(o_sb[:, t0:t0 + th], o_ps[:, 0:th])
                        nc.vector.tensor_copy(
                            o_sb[:, t0 + th:t0 + tn], o_ps[:, th:tn])
                    nc.sync.dma_start(
                        out=outT[hsl, bo:bo + wb], in_=o_sb[:])

    _enforce_single_wait(nc)
    return nc


def kernel(x, Wg, W1, W2, W3, _trace=False):
    global LAST_RESULTS
    xf = np.asarray(x, dtype=np.float32).reshape(-1, H)
    T = xf.shape[0]

    # --- Host router: top-2 + softmax over the selected pair (fp32) ---
    logits = xf @ np.asarray(Wg, dtype=np.float32)           # (T, E)
    top2 = np.argsort(-logits, axis=-1)[:, :K]               # (T, K)
    v = np.take_along_axis(logits, top2, axis=-1)
    m = v.max(axis=-1, keepdims=True)
    p = np.exp(v - m)
    rw = (p / p.sum(axis=-1, keepdims=True)).astype(np.float32)

    # --- Dispatch: tokens per expert; snake-pack experts into groups ---
    idx_e, wt_e = [], []
    for e in range(E):
        rows, slots = np.nonzero(top2 == e)
        idx_e.append(rows)
        wt_e.append(rw[rows, slots])
    counts = np.array([len(r) for r in idx_e])

    order = np.argsort(-counts, kind="stable")
    group_experts = [[] for _ in range(NG)]
    for j, e in enumerate(order):
        blk, k = j // NG, j % NG
        g = k if blk % 2 == 0 else NG - 1 - k
        group_experts[g].append(int(e))
    widths = tuple(
        max(8, int(-(-max(counts[group_experts[g][b]] for g in range(NG))
                     // 8) * 8))
        for b in range(M))
    t_tot = sum(widths)
    boffs = np.concatenate([[0], np.cumsum(widths)[:-1]]).astype(int)

    if widths not in _NC_CACHE:
        _NC_CACHE[widths] = _build_nc(widths)
    nc = _NC_CACHE[widths]

    in_maps = []
    for g in range(NG):
        xT_g = np.zeros((H, t_tot), dtype=ml_dtypes.bfloat16)
        for b, e in enumerate(group_experts[g]):
            xT_g[:, boffs[b]:boffs[b] + counts[e]] = \
                xf[idx_e[e]].T.astype(ml_dtypes.bfloat16)
        for s in range(M):
            sl = slice(s * I_LOC, (s + 1) * I_LOC)
            w1c = np.concatenate(
                [np.asarray(W1[e][:, sl], dtype=ml_dtypes.bfloat16)
                 for e in group_experts[g]], axis=1)
            w3c = np.concatenate(
                [np.asarray(W3[e][:, sl], dtype=ml_dtypes.bfloat16)
                 for e in group_experts[g]], axis=1)
            w2c = np.concatenate(
                [np.asarray(W2[e][sl, :], dtype=ml_dtypes.bfloat16)
                 for e in group_experts[g]], axis=0)
            in_maps.append({"xT": xT_g, "w1": w1c, "w3": w3c, "w2": w2c})

    res = run_bass_kernel_spmd(nc, in_maps, list(range(E)), trace=_trace)
    LAST_RESULTS = res

    # --- Combine: sum the M I-slice partials, weighted scatter-add ---
    out = np.zeros((T, H), dtype=np.float32)
    for g in range(NG):
        Y = np.zeros((H, t_tot), dtype=np.float32)
        for s in range(M):
            Y += np.asarray(res.results[g * M + s]["outT"],
                            dtype=np.float32)
        for b, e in enumerate(group_experts[g]):
            ne = counts[e]
            # rows are unique within one expert (top-2 indices distinct)
            out[idx_e[e]] += Y[:, boffs[b]:boffs[b] + ne].T * \
                wt_e[e][:, None]
    return out.reshape(np.asarray(x).shape).astype(np.float32)


# revision 11
# speedup vs baseline: 1.1583x; 1.1583x over previous
"""Mixtral sparse MoE block on 8 Trainium2 NeuronCores.

Strategy: grouped expert parallelism with intermediate-dim slicing.
Experts are snake-packed by token count into NG = 8//M groups of M
experts; each group owns M cores, and each core computes ALL M experts
of its group over an I/M slice of the intermediate dim. Block widths
(token counts per expert, position-wise max across groups, rounded to
8) are compile-time constants, so per-core work is Sum(widths) x
3*2*H*(I/M) FLOPs — balanced across cores and nearly padding-free —
while per-core weight DMA stays at 12.6 MB (same as plain expert
parallelism). The host runs the router / dispatch / combine (adds the
M partial outputs per group, then weighted scatter-add).

Device layout per core (features on partitions, tokens on free dim):
  up[i,t]   = sum_h W1[h,i] * xT[h,t]     i in this core's I/M slice
  gate[i,t] = sum_h W3[h,i] * xT[h,t]
  act[i,t]  = silu(up) * gate             (ACT silu + DVE mul -> bf16)
  out[h,t]  = sum_i W2[i,h] * act[i,t]    partial over the I/M slice

DMA queues: sync streams W1/W3 (+ x blocks 1..M-1) in consumption
order; gpsimd streams W2; scalar loads x block 0 then issues output
DMAs; vector only memsets the PE-warmup tile so warmup starts right
after the preamble instead of waiting on any DMA.
"""

import numpy as np
import ml_dtypes

import bass_rust
import concourse.bass as bass
import concourse.mybir as mybir
import concourse.tile as tile
from concourse.bass_utils import run_bass_kernel_spmd


def _enforce_single_wait(nc):
    """The walrus in this image rejects >1 sync-wait per instruction
    ("Too many sync wait commands", CoreV3GenImpl setupSyncWait). Hoist
    extra waits onto same-engine nops inserted just before the offender
    — waiting earlier on the same sequencer is always safe."""
    for f in nc.m.functions:
        for bb in f.blocks:
            insts = bb.instructions
            i = 0
            while i < len(insts):
                inst = insts[i]
                si = inst.sync_info
                if si is not None and len(si.on_wait) > 1:
                    waits = list(si.on_wait)
                    if any(w.wait_reg is not None for w in waits):
                        i += 1
                        continue
                    for j, w in enumerate(waits[:-1]):
                        nop = mybir.InstNoOp(
                            name=f"{inst.name}_hw{j}", ins=[], outs=[])
                        nop.engine = inst.engine
                        nop.sync_info = bass_rust.SyncInfo(
                            on_wait=[w], on_update=[])
                        insts.insert(i, nop)
                        i += 1
                    inst.sync_info = bass_rust.SyncInfo(
                        on_wait=[waits[-1]], on_update=list(si.on_update))
                i += 1

P = 128
H = 1024
I = 2048
E = 8
K = 2
M = 4            # I-dim slices per expert == experts per group
NG = E // M      # groups (sets of M cores)
I_LOC = I // M   # intermediate channels per core per expert
IT_LOC = I_LOC // P
HK = H // P
WARM_N = 14      # PE warmup matmuls (bridge preamble -> first real matmul)

BF16 = mybir.dt.bfloat16
F32 = mybir.dt.float32

# Populated by the last kernel() call so a harness can inspect HW timing.
LAST_RESULTS = None

_NC_CACHE = {}


def _t_chunks(w):
    """Split a block's token free-dim into matmul chunks <= 512 (PSUM bank)."""
    if w <= 512:
        return [(0, w)]
    half = (w + 1) // 2
    half = (half + 31) // 32 * 32
    return [(0, half), (half, w - half)]


def _build_nc(widths):
    """SPMD program: M expert blocks (token widths `widths`), each an
    I/M-slice SwiGLU MLP; runs on all 8 cores with per-core data."""
    t_tot = sum(widths)
    boffs = [0]
    for w in widths[:-1]:
        boffs.append(boffs[-1] + w)

    nc = bass.Bass()
    xT = nc.declare_dram_parameter("xT", [H, t_tot], BF16, isOutput=False)
    w1 = nc.declare_dram_parameter("w1", [H, I_LOC * M], BF16, isOutput=False)
    w3 = nc.declare_dram_parameter("w3", [H, I_LOC * M], BF16, isOutput=False)
    w2 = nc.declare_dram_parameter("w2", [I_LOC * M, H], BF16, isOutput=False)
    outT = nc.declare_dram_parameter("outT", [H, t_tot], BF16, isOutput=True)

    with tile.TileContext(nc) as tc:
        with (
            tc.tile_pool(name="x", bufs=1) as xpool,
            tc.tile_pool(name="wu", bufs=1) as wupool,
            tc.tile_pool(name="wg", bufs=1) as wgpool,
            tc.tile_pool(name="wd", bufs=1) as wdpool,
            tc.tile_pool(name="acts", bufs=1) as actpool,
            tc.tile_pool(name="warm", bufs=1) as warmpool,
            tc.tile_pool(name="ost", bufs=1) as ostpool,
            tc.tile_pool(name="ps", bufs=2, space="PSUM") as pspool,
            tc.tile_pool(name="ev", bufs=3) as evpool,
        ):
            # Warmup tile: memset on the (otherwise idle) vector queue so
            # PE warmup needs no DMA and starts right after the preamble.
            warm_sb = warmpool.tile([P, 512], BF16, tag="warm", name="warm")
            nc.vector.memset(warm_sb[:], 0.0)

            x_sb = [xpool.tile([P, t_tot], BF16, tag=f"x{hk}", name=f"x{hk}")
                    for hk in range(HK)]
            w1_sb = [wupool.tile([P, I_LOC * M], BF16, tag=f"w1_{hk}",
                                 name=f"w1_{hk}") for hk in range(HK)]
            w3_sb = [wgpool.tile([P, I_LOC * M], BF16, tag=f"w3_{hk}",
                                 name=f"w3_{hk}") for hk in range(HK)]
            w2_sb = [wdpool.tile([P, H], BF16, tag=f"w2_{it}", name=f"w2_{it}")
                     for it in range(IT_LOC * M)]

            # Deadline-ordered streaming. HBM aggregate (~360 GB/s) binds,
            # so only block-0 prerequisites move first: x block 0 on
            # scalar, W1 block 0 on sync, W3 block 0 on gpsimd (~1 MB per
            # queue, 1 KB lines). Then sync streams the W1/W3 bundles for
            # blocks 1..M-1 while gpsimd interleaves x and W2 by deadline.
            b0 = slice(boffs[0], boffs[0] + widths[0])
            for hk in range(HK):
                hs = slice(hk * P, (hk + 1) * P)
                nc.scalar.dma_start(out=x_sb[hk][:, b0], in_=xT[hs, b0])
            cs = slice(0, I_LOC)
            for hk in range(HK):
                hs = slice(hk * P, (hk + 1) * P)
                nc.sync.dma_start(out=w1_sb[hk][:, cs], in_=w1[hs, cs])
            for hk in range(HK):
                hs = slice(hk * P, (hk + 1) * P)
                nc.gpsimd.dma_start(out=w3_sb[hk][:, cs], in_=w3[hs, cs])

            for b in range(1, M):
                cs = slice(b * I_LOC, (b + 1) * I_LOC)
                for hk in range(HK):
                    hs = slice(hk * P, (hk + 1) * P)
                    nc.sync.dma_start(out=w1_sb[hk][:, cs], in_=w1[hs, cs])
                for hk in range(HK):
                    hs = slice(hk * P, (hk + 1) * P)
                    nc.sync.dma_start(out=w3_sb[hk][:, cs], in_=w3[hs, cs])

            # gpsimd: x block b arrives well before phase A(b); W2 block b
            # well before phase B(b).
            for b in range(1, M):
                bs = slice(boffs[b], boffs[b] + widths[b])
                for hk in range(HK):
                    hs = slice(hk * P, (hk + 1) * P)
                    nc.gpsimd.dma_start(out=x_sb[hk][:, bs], in_=xT[hs, bs])
                for it in range((b - 1) * IT_LOC, b * IT_LOC):
                    nc.gpsimd.dma_start(
                        out=w2_sb[it][:], in_=w2[it * P:(it + 1) * P, :])
            for it in range((M - 1) * IT_LOC, M * IT_LOC):
                nc.gpsimd.dma_start(
                    out=w2_sb[it][:], in_=w2[it * P:(it + 1) * P, :])

            act_sb = [[actpool.tile([P, widths[b]], BF16, tag=f"a{b}_{it}",
                                    name=f"a{b}_{it}")
                       for it in range(IT_LOC)] for b in range(M)]

            # PE warmup on the memset tile: flips the HAM clock gate to 8/8
            # and bridges until the first weights/x arrive (~11us).
            for wi in range(WARM_N):
                w_ps = pspool.tile([P, 512], F32, tag="warm", bufs=1,
                                   name=f"warm{wi}")
                nc.tensor.matmul(
                    w_ps[:], warm_sb[:, 0:P], warm_sb[:],
                    start=True, stop=True)

            for b in range(M):
                wb = widths[b]
                bo = boffs[b]
                chunks = _t_chunks(wb)
                # Phase A: up/gate matmuls + fused silu*gate eviction.
                for it in range(IT_LOC):
                    isl = slice(b * I_LOC + it * P, b * I_LOC + (it + 1) * P)
                    for (t0, tn) in chunks:
                        tsl = slice(bo + t0, bo + t0 + tn)
                        up_ps = pspool.tile([P, tn], F32, tag="up",
                                            name=f"up{b}_{it}_{t0}")
                        gt_ps = pspool.tile([P, tn], F32, tag="gt",
                                            name=f"gt{b}_{it}_{t0}")
                        for hk in range(HK):
                            nc.tensor.matmul(
                                up_ps[:], w1_sb[hk][:, isl], x_sb[hk][:, tsl],
                                start=(hk == 0), stop=(hk == HK - 1))
                        for hk in range(HK):
                            nc.tensor.matmul(
                                gt_ps[:], w3_sb[hk][:, isl], x_sb[hk][:, tsl],
                                start=(hk == 0), stop=(hk == HK - 1))
                        silu_t = evpool.tile([P, tn], F32, tag="silu",
                                             name=f"silu{b}_{it}_{t0}")
                        nc.scalar.activation(
                            silu_t[:], up_ps[:],
                            mybir.ActivationFunctionType.Silu)
                        nc.vector.tensor_mul(
                            act_sb[b][it][:, t0:t0 + tn], silu_t[:], gt_ps[:])

                # Phase B: down projection; each o_ps is evicted by scalar
                # and vector in parallel halves (bf16) into a dedicated
                # per-(block,h) staging tile (bufs=HK: no reuse, so a
                # backed-up output queue can never stall the PE), then
                # DMA'd from gpsimd (last block: scalar, for tail latency).
                for h in range(HK):
                    hsl = slice(h * P, (h + 1) * P)
                    o_sb = ostpool.tile([P, wb], BF16, tag=f"ost{b}", bufs=HK,
                                        name=f"ost{b}_{h}")
                    for (t0, tn) in chunks:
                        o_ps = pspool.tile([P, tn], F32, tag="o", bufs=3,
                                           name=f"o{b}_{h}_{t0}")
                        for it in range(IT_LOC):
                            nc.tensor.matmul(
                                o_ps[:],
                                w2_sb[b * IT_LOC + it][:, hsl],
                                act_sb[b][it][:, t0:t0 + tn],
                                start=(it == 0), stop=(it == IT_LOC - 1))
                        th = (tn + 15) // 16 * 8
                        nc.scalar.copy(o_sb[:, t0:t0 + th], o_ps[:, 0:th])
                        nc.vector.tensor_copy(
                            o_sb[:, t0 + th:t0 + tn], o_ps[:, th:tn])
                    out_eng = nc.scalar if b == M - 1 else nc.gpsimd
                    out_eng.dma_start(
                        out=outT[hsl, bo:bo + wb], in_=o_sb[:])

    _enforce_single_wait(nc)
    return nc


def kernel(x, Wg, W1, W2, W3, _trace=False):
    global LAST_RESULTS
    xf = np.asarray(x, dtype=np.float32).reshape(-1, H)
    T = xf.shape[0]

    # --- Host router: top-2 + softmax over the selected pair (fp32) ---
    logits = xf @ np.asarray(Wg, dtype=np.float32)           # (T, E)
    top2 = np.argsort(-logits, axis=-1)[:, :K]               # (T, K)
    v = np.take_along_axis(logits, top2, axis=-1)
    m = v.max(axis=-1, keepdims=True)
    p = np.exp(v - m)
    rw = (p / p.sum(axis=-1, keepdims=True)).astype(np.float32)

    # --- Dispatch: tokens per expert; snake-pack experts into groups ---
    idx_e, wt_e = [], []
    for e in range(E):
        rows, slots = np.nonzero(top2 == e)
        idx_e.append(rows)
        wt_e.append(rw[rows, slots])
    counts = np.array([len(r) for r in idx_e])

    order = np.argsort(-counts, kind="stable")
    group_experts = [[] for _ in range(NG)]
    for j, e in enumerate(order):
        blk, k = j // NG, j % NG
        g = k if blk % 2 == 0 else NG - 1 - k
        group_experts[g].append(int(e))
    widths = tuple(
        max(8, int(-(-max(counts[group_experts[g][b]] for g in range(NG))
                     // 8) * 8))
        for b in range(M))
    t_tot = sum(widths)
    boffs = np.concatenate([[0], np.cumsum(widths)[:-1]]).astype(int)

    if widths not in _NC_CACHE:
        _NC_CACHE[widths] = _build_nc(widths)
    nc = _NC_CACHE[widths]

    in_maps = []
    for g in range(NG):
        xT_g = np.zeros((H, t_tot), dtype=ml_dtypes.bfloat16)
        for b, e in enumerate(group_experts[g]):
            xT_g[:, boffs[b]:boffs[b] + counts[e]] = \
                xf[idx_e[e]].T.astype(ml_dtypes.bfloat16)
        for s in range(M):
            sl = slice(s * I_LOC, (s + 1) * I_LOC)
            w1c = np.concatenate(
                [np.asarray(W1[e][:, sl], dtype=ml_dtypes.bfloat16)
                 for e in group_experts[g]], axis=1)
            w3c = np.concatenate(
                [np.asarray(W3[e][:, sl], dtype=ml_dtypes.bfloat16)
                 for e in group_experts[g]], axis=1)
            w2c = np.concatenate(
                [np.asarray(W2[e][sl, :], dtype=ml_dtypes.bfloat16)
                 for e in group_experts[g]], axis=0)
            in_maps.append({"xT": xT_g, "w1": w1c, "w3": w3c, "w2": w2c})

    res = run_bass_kernel_spmd(nc, in_maps, list(range(E)), trace=_trace)
    LAST_RESULTS = res

    # --- Combine: sum the M I-slice partials, weighted scatter-add ---
    out = np.zeros((T, H), dtype=np.float32)
    for g in range(NG):
        Y = np.zeros((H, t_tot), dtype=np.float32)
        for s in range(M):
            Y += np.asarray(res.results[g * M + s]["outT"],
                            dtype=np.float32)
        for b, e in enumerate(group_experts[g]):
            ne = counts[e]
            # rows are unique within one expert (top-2 indices distinct)
            out[idx_e[e]] += Y[:, boffs[b]:boffs[b] + ne].T * \
                wt_e[e][:, None]
    return out.reshape(np.asarray(x).shape).astype(np.float32)
